# revision 3
# baseline (speedup 1.0000x reference)
"""Dot-product attention (no softmax) on 8 TRN2 NeuronCores.

out[b,h] = (q[b,h] @ k[b,h].T) @ v[b,h]  for q,k,v [B,H,L,D] = [2,16,2048,64] f32.

Strategy: matmul associativity -> out = q @ (k.T @ v). KV = k.T@v is [64,64]
per head, so the problem collapses from O(L^2 D) to O(L D^2) flops and becomes
purely memory bound (48 MiB in / 16 MiB out).

Sharding: the 32 (b,h) attention instances are independent; each of the 8
cores handles 4 consecutive heads of the flattened (b*h) axis. No collectives.

v2 changes vs the 41.5us baseline (trace-driven):
- fp32 matmuls cost 4 cycles/row on the PE (transposes 2) vs 1 for bf16; the
  baseline was Tensor-serialized (31us PE busy > 25us DMA busy). All compute
  inputs are now bf16: the load DMAs cast f32->bf16 inline (SWDGE path), which
  is free in the DMA datapath and keeps HBM traffic unchanged. PSUM stays f32
  and the output is stored as f32, so only operand rounding is lost (~1e-3
  rel err, tolerance is 2e-2).
- q,k,v are staged host-side into one [HPC, 3, L, D] tensor so each head is a
  single 1.5 MiB cast-DMA (4 load instructions instead of 12; DMA_DIRECT2D
  issue cost was ~0.6us each on the baseline's sync queue).
- Small memsets moved off the gpsimd queue (vector engine) so SWDGE issue is
  not delayed behind them.

Per-core layout trick: a head's [2048, 64] tensor is viewed as [128, 16, 64]
(partition p holds rows 16p..16p+15, 4 KiB contiguous DRAM per partition, so
every DMA is fully coalesced). The KV reduction over L is order-independent,
and the same interleaved row mapping flows through transpose -> matmul ->
store unchanged.
"""

import sys

if "/opt/trn_rl_repo" not in sys.path:
    sys.path.insert(0, "/opt/trn_rl_repo")

from contextlib import ExitStack

import numpy as np

import concourse.bass as bass
import concourse.tile as tile
from concourse import bacc, mybir
from concourse.bass_utils import run_bass_kernel_spmd

B, H, L, D = 2, 16, 2048, 64
N_CORES = 8
HPC = (B * H) // N_CORES  # heads per core = 4
P = 128
J = L // P  # 16 row-slots per partition
F32 = mybir.dt.float32
BF16 = mybir.dt.bfloat16


def _body(ctx: ExitStack, tc: tile.TileContext, o_d, qkv_d):
    nc = tc.nc

    const_pool = ctx.enter_context(tc.tile_pool(name="const", bufs=1))
    in_pool = ctx.enter_context(tc.tile_pool(name="in", bufs=4))
    qt_pool = ctx.enter_context(tc.tile_pool(name="qt", bufs=32))
    kv_pool = ctx.enter_context(tc.tile_pool(name="kv", bufs=4))
    out_pool = ctx.enter_context(tc.tile_pool(name="out", bufs=4))
    psum_kv = ctx.enter_context(tc.tile_pool(name="psum_kv", bufs=1, space="PSUM"))
    psum_s = ctx.enter_context(tc.tile_pool(name="psum_s", bufs=1, space="PSUM"))
    psum_t = ctx.enter_context(tc.tile_pool(name="psum_t", bufs=3, space="PSUM"))
    psum_o = ctx.enter_context(tc.tile_pool(name="psum_o", bufs=3, space="PSUM"))

    # Per-head fused qkv tile: [128, 3(q/k/v), 16, 64] bf16 (6 KiB/partition).
    qkv_sbs = [
        in_pool.tile([P, 3, J, D], BF16, tag="qkv", name=f"qkv{h}") for h in range(HPC)
    ]

    def qkv_view(h):
        # [3, L, D] f32 in DRAM -> [p, t, j, d]; per partition 3 chunks of
        # 4 KiB (q/k/v), fully coalesced descriptors.
        return qkv_d[h].rearrange("t (p j) d -> p t j d", p=P)

    # First ops on the gpsimd queue: the cast-loads themselves (SWDGE is the
    # only DMA path that can cast f32->bf16 inline). Head 0's q slab pair goes
    # first so the transposes have work as early as possible.
    nc.gpsimd.dma_start(qkv_sbs[0][:, 0], qkv_view(0)[:, 0])  # q0
    nc.gpsimd.dma_start(qkv_sbs[0][:, 1:3], qkv_view(0)[:, 1:3])  # k0, v0
    for h in range(1, HPC):
        nc.gpsimd.dma_start(qkv_sbs[h][:], qkv_view(h))

    # HAM warm-up: ~3.4us of dense bf16 matmuls while the first DMAs are in
    # flight, so the PE clock un-throttles (4/8 -> 8/8) before the real
    # transposes/matmuls start. Results are never read.
    warm_in = const_pool.tile([P, 4 * P], BF16)
    nc.vector.memset(warm_in[:], 0.0)
    warm_ps = psum_o.tile([P, 4 * P], F32, tag="o_ps", name="warm_ps")
    for _ in range(8):
        nc.tensor.matmul(
            warm_ps[:], warm_in[:, 0:P], warm_in[:], start=True, stop=True
        )

    # Identity (bf16) for PE transposes.
    ident = const_pool.tile([P, P], BF16)
    nc.vector.memset(ident[:], 0.0)
    nc.gpsimd.affine_select(
        out=ident[:],
        in_=ident[:],
        compare_op=mybir.AluOpType.not_equal,
        fill=1.0,
        base=0,
        pattern=[[-1, P]],
        channel_multiplier=1,
    )

    # ones_dbl[p, m] = 1 iff p == m (mod 64): one matmul against it both sums
    # the two column-tiled KV halves and replicates the result to partitions
    # 64..127 (needed as the row-group-1 operand of the row-tiled out matmuls).
    ones_dbl = const_pool.tile([P, P], BF16)
    nc.vector.memset(ones_dbl[:], 0.0)
    for off in (-64, 0, 64):
        nc.gpsimd.affine_select(
            out=ones_dbl[:],
            in_=ones_dbl[:],
            compare_op=mybir.AluOpType.not_equal,
            fill=1.0,
            base=-off,
            pattern=[[-1, P]],
            channel_multiplier=1,
        )

    # Software-pipelined emission: every head's transpose + KV + KV2 chain is
    # emitted before any O phase, so the cross-engine kv2 chains (PSUM copy ->
    # ones_dbl matmul -> kv2 copies) hide under other heads' PE work instead
    # of stalling it — in particular the last head's chain is not exposed at
    # the kernel tail.
    qts_all, kv2s = [], []
    for h in range(HPC):
        q_sb = qkv_sbs[h][:, 0]
        k_sb = qkv_sbs[h][:, 1]
        v_sb = qkv_sbs[h][:, 2]

        qts = []
        for jp in range(J // 2):
            qt_ps = psum_t.tile([P, P], BF16, tag="qt_ps")
            nc.tensor.transpose(qt_ps[:], q_sb[:, 2 * jp : 2 * jp + 2], ident[:])
            qt_sb = qt_pool.tile([P, P], BF16, tag="qt", name=f"qt{h}_{jp}")
            nc.scalar.activation(
                qt_sb[:], qt_ps[:], mybir.ActivationFunctionType.Identity
            )
            qts.append(qt_sb)
        qts_all.append(qts)

        # KV = k.T @ v, column-tiled: even j-slots accumulate into PE columns
        # 0..63 (psum partitions 0..63), odd slots into columns 64..127, so
        # the two matmuls of a pair run concurrently in the array.
        kv_ps = psum_kv.tile([P, D], F32)
        for jp in range(J // 2):
            nc.tensor.matmul(
                kv_ps[0:D],
                k_sb[:, 2 * jp],
                v_sb[:, 2 * jp],
                start=(jp == 0),
                stop=(jp == J // 2 - 1),
                tile_position=(0, 0),
                skip_group_check=True,
            )
            nc.tensor.matmul(
                kv_ps[D : 2 * D],
                k_sb[:, 2 * jp + 1],
                v_sb[:, 2 * jp + 1],
                start=(jp == 0),
                stop=(jp == J // 2 - 1),
                tile_position=(0, D),
                skip_group_check=True,
            )
        kv_raw = kv_pool.tile([P, D], BF16, tag="kv_raw", name=f"kvr{h}")
        nc.vector.tensor_copy(kv_raw[:], kv_ps[:])
        kv_st_ps = psum_s.tile([P, D], F32, tag="kv_st", name=f"kvs{h}")
        nc.tensor.matmul(kv_st_ps[:], ones_dbl[:], kv_raw[:], start=True, stop=True)
        # KV2 = blockdiag(KV, KV): one [128,128] matmul against it computes two
        # output slots at once (lhsT = a transposed q slab pair).
        kv2 = kv_pool.tile([P, P], BF16, tag="kv2", name=f"kv2_{h}")
        nc.vector.memset(kv2[:], 0.0)
        nc.vector.tensor_copy(kv2[0:D, 0:D], kv_st_ps[0:D])
        nc.vector.tensor_copy(kv2[D : 2 * D, D : 2 * D], kv_st_ps[D : 2 * D])
        kv2s.append(kv2)

    for h in range(HPC):
        out_sb = out_pool.tile([P, J, D], F32, tag="o", name=f"o{h}")
        ov = o_d[h].rearrange("(p j) d -> p j d", p=P)
        kv2 = kv2s[h]
        for jp in range(J // 2):
            o_ps = psum_o.tile([P, 2, D], F32, tag="o_ps")
            nc.tensor.matmul(o_ps[:], qts_all[h][jp][:], kv2[:], start=True, stop=True)
            nc.vector.tensor_copy(out_sb[:, 2 * jp : 2 * jp + 2], o_ps[:])
            if h == HPC - 1:
                # last head: store per pair-of-slots so the ~2us HBM
                # completion receipts of the final DMAs overlap
                if jp % 2 == 1:
                    sl = slice(2 * jp - 2, 2 * jp + 2)
                    nc.sync.dma_start(ov[:, sl], out_sb[:, sl])
            elif jp == J // 4 - 1:
                nc.sync.dma_start(ov[:, 0 : J // 2], out_sb[:, 0 : J // 2])
        if h != HPC - 1:
            nc.sync.dma_start(ov[:, J // 2 : J], out_sb[:, J // 2 : J])


def build():
    nc = bacc.Bacc("TRN2", target_bir_lowering=False, debug=False)
    qkv_d = nc.dram_tensor("qkv", [HPC, 3, L, D], F32, kind="ExternalInput").ap()
    o_d = nc.dram_tensor("out", [HPC, L, D], F32, kind="ExternalOutput").ap()
    with tile.TileContext(nc) as tc, ExitStack() as ctx:
        _body(ctx, tc, o_d, qkv_d)
    nc.compile()
    return nc


_NC = None


def _get_nc():
    global _NC
    if _NC is None:
        _NC = build()
    return _NC


def make_in_maps(q, k, v):
    qf = np.asarray(q, dtype=np.float32).reshape(B * H, L, D)
    kf = np.asarray(k, dtype=np.float32).reshape(B * H, L, D)
    vf = np.asarray(v, dtype=np.float32).reshape(B * H, L, D)
    # [B*H, 3, L, D]: per head q/k/v adjacent so one DMA loads a whole head.
    qkv = np.stack([qf, kf, vf], axis=1)
    return [
        {"qkv": np.ascontiguousarray(qkv[c * HPC : (c + 1) * HPC])}
        for c in range(N_CORES)
    ]


def run_sharded(q, k, v, **spmd_kwargs):
    """Run on all 8 cores; returns (full_output, BassKernelResults)."""
    nc = _get_nc()
    res = run_bass_kernel_spmd(
        nc, make_in_maps(q, k, v), core_ids=list(range(N_CORES)), **spmd_kwargs
    )
    shards = [np.asarray(res.results[c]["out"]) for c in range(N_CORES)]
    out = np.concatenate(shards, axis=0).reshape(B, H, L, D).astype(np.float32)
    return out, res


def kernel(q, k, v):
    out, _ = run_sharded(q, k, v)
    return out
